# revision 16
# baseline (speedup 1.0000x reference)
"""TRN2 Bass kernel for nn_CIE_48052094108098 (sparse_attention).

Model (S=2048, B=4, D=512, H=8 -> HH=4 heads/module, DH=128):
  gates = sigmoid(MLP([mean(x[:1024]), mean(x[1024:]), |diff|]))   (per batch)
  xn = LayerNorm(x)
  homo-MHA: same-half block attention, v gated by gates[:,0]
  het-MHA:  cross-half block attention, v gated by gates[:,1]
  y = x + homo + het;  out = y + FFN(y)   (GELU exact)

Sharding: 8 cores = (batch b) x (query half p). Each core computes the full
output for its (b, half-p) rows; the block masks make each attention dense
over 1024-wide blocks. Zero cross-core communication.

v4: fp8(e4m3) DoubleRow matmuls for QKV projections, AV, softmax
denominators, out-proj and FFN mm1; scores bf16; FFN mm2 bf16 (the hT/w2
fp8 quantization dominated the error budget). Reductions (LN stats,
softmax den) use an all-ones [128,2,128] fp8 stationary so results land in
PSUM broadcast across partitions. LN critical path: half-0 stats first,
half-1 squares on gpsimd, rstd = exp(-0.5*ln(var+eps)) so the scalar
engine stays on one ACT table-set (ln+exp) until GELU. Attention inner
loop is software-pipelined: the next tile-pair's score matmuls are emitted
before the previous pair's AV/den so the PE never waits on the exp.
Module-t projections run inside module-h's attention (PE slack); the
scalar engine never idles between the two modules' exp phases.
"""
import sys

sys.path.insert(0, "/opt/trn_rl_repo")

import numpy as np

S, B, D = 2048, 4, 512
SH = S // 2          # 1024 (half)
HH, DH = 4, 128      # heads per module, head dim
HID = 128            # gater hidden
FF = 4 * D           # 2048
NCORE = 8
ND = D // 128        # 4 feature tiles
NFF = FF // 128      # 16
NT = SH // 128       # 8 t-tiles per kv half
NSQ = SH // 512      # 2 query s-chunks

_CACHED = {}


def build_nc():
    import concourse.mybir as mybir
    import concourse.tile as tile
    from concourse import bacc

    F32 = mybir.dt.float32
    BF16 = mybir.dt.bfloat16
    F8 = mybir.dt.float8e4
    ACTF = mybir.ActivationFunctionType
    ALU = mybir.AluOpType
    DRM = mybir.MatmulPerfMode.DoubleRow

    nc = bacc.Bacc("TRN2", target_bir_lowering=False, debug=False,
                   num_devices=NCORE)

    # ---- DRAM parameters ----
    dp = nc.declare_dram_parameter
    xT_d = dp("xT", [D, S], BF16, isOutput=False)            # cols: [query half | other half]
    ones128b_d = dp("ones128b", [128, 128], BF16, isOutput=False)
    gw1T_d = dp("gw1T", [3 * D, HID], F32, isOutput=False)  # row blocks: f_q, f_o, |diff| (x 1/SH)
    gb1_d = dp("gb1", [HID, 1], F32, isOutput=False)
    gw2T_d = dp("gw2T", [HID, 2], F32, isOutput=False)      # col 0: homo gate, col 1: het
    gb2n_d = dp("gb2n", [1, 2], F32, isOutput=False)        # NEGATED gb2 (for exp-sigmoid)
    wqT_d, wkT_d, wvT_d, bqk_d, wbv_d, woT_d = {}, {}, {}, {}, {}, {}
    for m in ("h", "t"):
        wqT_d[m] = dp(f"wqT_{m}", [D, D], F8, isOutput=False)
        wkT_d[m] = dp(f"wkT_{m}", [D, D], F8, isOutput=False)
        wvT_d[m] = dp(f"wvT_{m}", [D, D], F8, isOutput=False)
        bqk_d[m] = dp(f"bqk_{m}", [2 * D], F32, isOutput=False)   # [bq(512) | bk(512)]
        wbv_d[m] = dp(f"wbv_{m}", [D], F32, isOutput=False)       # out_w @ v_bias
        woT_d[m] = dp(f"woT_{m}", [D, D], F8, isOutput=False)
    bo_d = dp("bo", [D], F32, isOutput=False)                # homo_out_b + het_out_b
    w1T_d = dp("w1T", [D, FF], F8, isOutput=False)
    b1_d = dp("b1", [FF], F32, isOutput=False)
    w2T_d = dp("w2T", [FF, D], BF16, isOutput=False)
    b2_d = dp("b2", [D], F32, isOutput=False)
    ones128f8_d = dp("ones128f8", [128, 2, 128], F8, isOutput=False)
    xq32_d = dp("xq32", [D, SH], F32, isOutput=False)
    zT_d = dp("zT", [D, SH], BF16, isOutput=True)

    INV_SQRT_DH = float(1.0 / np.sqrt(DH))

    def re3(ap):    # [*,N] DRAM -> [128, n, N] partition-tiled view
        return ap.rearrange("(n p) f -> p n f", p=128)

    def rev(ap):    # [N] DRAM vector -> [128, n] column view
        return ap.rearrange("(n p) -> p n", p=128)

    lp = nc.allow_low_precision("fp8/bf16 intermediates: tolerance is 2e-2")
    lp.__enter__()
    with tile.TileContext(nc, pool_alloc_mode="queue") as tc:
        const = tc.alloc_tile_pool(name="const", bufs=1)
        big = tc.alloc_tile_pool(name="big", bufs=1)
        ppt = tc.alloc_tile_pool(name="ppt", bufs=7)
        pbc = tc.alloc_tile_pool(name="pbc", bufs=2)
        psmall = tc.alloc_tile_pool(name="psmall", bufs=16)
        pz = tc.alloc_tile_pool(name="pz", bufs=4)
        psum = tc.alloc_tile_pool(name="psum", bufs=2, space="PSUM")

        def ps_big2(nm):       # [128,1024] two-bank psum (pairs of 512-chunks)
            return psum.tile([128, 1024], F32, name=nm, tag="big2", bufs=2)

        def ps_av2(nm):
            return psum.tile([128, 1024], F32, name=nm, tag="av2", bufs=2)

        # ---------- constants ----------
        # x first: LN is the first consumer of DMA data
        xt_q = big.tile([128, ND, SH], BF16, tag="t_xq")
        xt_o = big.tile([128, ND, SH], BF16, tag="t_xo")
        for d in range(ND):
            nc.sync.dma_start(
                out=xt_q[:, d, :],
                in_=xT_d[d * 128:(d + 1) * 128, 0:SH])
        ones128f8_t = const.tile([128, 2, 128], F8)
        nc.sync.dma_start(out=ones128f8_t, in_=ones128f8_d[:, :, :])
        ones128b_t = const.tile([128, 128], BF16)
        nc.sync.dma_start(out=ones128b_t, in_=ones128b_d[:, :])
        for d in range(ND):
            nc.sync.dma_start(
                out=xt_o[:, d, :],
                in_=xT_d[d * 128:(d + 1) * 128, SH:S])
        gw1T_t = const.tile([128, 12, HID], F32)
        nc.sync.dma_start(out=gw1T_t, in_=gw1T_d[:, :].rearrange("(n p) h -> p n h", p=128))
        gb1_t = const.tile([128, 1], F32)
        nc.sync.dma_start(out=gb1_t, in_=gb1_d[:, :])
        gw2T_t = const.tile([128, 2], F32)
        nc.sync.dma_start(out=gw2T_t, in_=gw2T_d[:, :])
        gb2n_t = const.tile([1, 2], F32)
        nc.sync.dma_start(out=gb2n_t, in_=gb2n_d[:, :])
        bqk_t, wbv_t = {}, {}
        for m in ("h", "t"):
            bqk_t[m] = const.tile([128, 8], F32, name=f"bqk_t_{m}", tag=f"bqk_{m}")
            nc.sync.dma_start(out=bqk_t[m], in_=rev(bqk_d[m][:]))
            wbv_t[m] = const.tile([128, ND], F32, name=f"wbv_t_{m}", tag=f"wbv_{m}")
            nc.sync.dma_start(out=wbv_t[m], in_=rev(wbv_d[m][:]))
        bo_t = const.tile([128, ND], F32)
        nc.sync.dma_start(out=bo_t, in_=rev(bo_d[:]))
        b1_t = const.tile([128, NFF], F32)
        nc.sync.dma_start(out=b1_t, in_=rev(b1_d[:]))
        b2_t = const.tile([128, ND], F32)
        nc.sync.dma_start(out=b2_t, in_=rev(b2_d[:]))
        eps_t = const.tile([128, 1], F32)
        nc.vector.memset(eps_t, 1e-5)
        zero_t = const.tile([128, 1], F32)
        nc.vector.memset(zero_t, 0.0)

        def load_qkv(m, tags):
            wq = big.tile([128, ND, D], F8, name=f"wq_{m}", tag=tags[0])
            nc.sync.dma_start(out=wq, in_=re3(wqT_d[m][:, :]))
            wk = big.tile([128, ND, D], F8, name=f"wk_{m}", tag=tags[1])
            nc.sync.dma_start(out=wk, in_=re3(wkT_d[m][:, :]))
            wv = big.tile([128, ND, D], F8, name=f"wv_{m}", tag=tags[2])
            nc.sync.dma_start(out=wv, in_=re3(wvT_d[m][:, :]))
            return wq, wk, wv

        w_qkv = {"h": load_qkv("h", ("t_wa", "t_wb", "t_wc"))}

        xn = big.tile([128, ND, S], F8, tag="t_xn")

        # ---------- LN half-0 stats (critical path for the projections) ----------
        xsq0 = []
        for pp in range(2):
            sq2 = ppt.tile([128, 2, 1024], F8, name=f"xsq0_{pp}", tag="pt")
            for i in range(2):
                nc.scalar.activation(sq2[:, i, :], xt_q[:, 2 * pp + i, :], ACTF.Square)
            xsq0.append(sq2)
        sumB0 = ps_av2("lnsum0")
        for d in range(ND):
            for c in range(2):
                nc.tensor.matmul(sumB0[:, c * 512:(c + 1) * 512], ones128b_t,
                                 xt_q[:, d, c * 512:(c + 1) * 512],
                                 start=(d == 0), stop=(d == ND - 1))
        ssqB0 = ps_av2("lnssq0")
        for pp in range(2):
            for c in range(2):
                nc.tensor.matmul(ssqB0[:, c * 512:(c + 1) * 512], ones128f8_t,
                                 xsq0[pp][:, :, c * 512:(c + 1) * 512],
                                 start=(pp == 0), stop=(pp == 1), perf_mode=DRM)
        # half-1 squares on gpsimd (off the critical path); sum now, ssq later
        xsq1 = []
        for pp in range(2):
            sq2 = ppt.tile([128, 2, 1024], F8, name=f"xsq1_{pp}", tag="pt")
            for i in range(2):
                nc.gpsimd.tensor_mul(sq2[:, i, :], xt_o[:, 2 * pp + i, :],
                                     xt_o[:, 2 * pp + i, :])
            xsq1.append(sq2)
        sumB1 = ps_big2("lnsum1")
        for d in range(ND):
            for c in range(2):
                nc.tensor.matmul(sumB1[:, c * 512:(c + 1) * 512], ones128b_t,
                                 xt_o[:, d, c * 512:(c + 1) * 512],
                                 start=(d == 0), stop=(d == ND - 1))

        def row_math(hf, sumB, ssqB):
            meanB = pbc.tile([128, 1024], BF16, name=f"meanB{hf}", tag="lnB", bufs=4)
            nc.vector.tensor_scalar_mul(meanB, sumB, 1.0 / D)
            msq = ppt.tile([128, 1024], F32, name=f"msq{hf}", tag="pt")
            nc.vector.tensor_mul(msq, meanB, meanB)
            var = ppt.tile([128, 1024], F32, name=f"var{hf}", tag="pt")
            nc.vector.scalar_tensor_tensor(out=var, in0=ssqB, scalar=1.0 / D, in1=msq,
                                           op0=ALU.mult, op1=ALU.subtract)
            std = ppt.tile([128, 1024], F32, name=f"std{hf}", tag="pt")
            nc.scalar.activation(std, var, ACTF.Sqrt, bias=eps_t)
            rstdB = pbc.tile([128, 1024], F32, name=f"rstdB{hf}", tag="lnB", bufs=4)
            nc.vector.reciprocal_approx_fast(rstdB, std)
            return rstdB, meanB

        rstdB0, meanB0 = row_math(0, sumB0, ssqB0)
        # apply, q half (DVE; chunked so V0-3 can start after chunk 0)
        for c in range(2):
            for d in range(ND):
                t1 = ppt.tile([128, 512], BF16, name=f"lnt0_{d}{c}", tag="pth", bufs=4)
                nc.vector.tensor_sub(t1, xt_q[:, d, c * 512:(c + 1) * 512],
                                     meanB0[:, c * 512:(c + 1) * 512])
                nc.vector.tensor_mul(xn[:, d, c * 512:(c + 1) * 512], t1,
                                     rstdB0[:, c * 512:(c + 1) * 512])

        gates, gate128 = {}, {}
        bo_eff_box = {}

        def gater_tail():
            g1_psum = psum.tile([128, 1], F32, name="g1_psum", tag="big2", bufs=2)
            for i in range(12):
                nc.tensor.matmul(g1_psum, gw1T_t[:, i, :], g_in[i],
                                 start=(i == 0), stop=(i == 11))
            relu_t = psmall.tile([128, 1], F32, tag="gsm")
            nc.vector.scalar_tensor_tensor(out=relu_t, in0=g1_psum, scalar=gb1_t,
                                           in1=zero_t, op0=ALU.add, op1=ALU.max)
            for j, m in enumerate(("h", "t")):
                g2_psum = psum.tile([1, 1], F32, name=f"g2_psum{j}", tag="big2", bufs=2)
                nc.tensor.matmul(g2_psum, gw2T_t[:, j:j + 1], relu_t, start=True, stop=True)
                # sigmoid without an ACT table switch: exp (already loaded) + recip
                e_t = psmall.tile([1, 1], F32, name=f"ge{j}", tag="gsm")
                nc.scalar.activation(e_t, g2_psum, ACTF.Exp, scale=-1.0,
                                     bias=gb2n_t[:, j:j + 1])
                e1_t = psmall.tile([1, 1], F32, name=f"ge1{j}", tag="gsm")
                nc.vector.tensor_scalar_add(e1_t, e_t, 1.0)
                gate = psmall.tile([1, 1], F32, name=f"gate{j}", tag="gsm")
                nc.vector.reciprocal_approx_fast(gate, e1_t)
                gates[m] = gate
                g128 = pbc.tile([128, 1], F32, name=f"g128_{j}", tag="g128", bufs=2)
                nc.gpsimd.partition_broadcast(g128, gate)
                gate128[m] = g128
            bo1 = psmall.tile([128, ND], F32, name="bo1", tag="boe", bufs=2)
            nc.vector.scalar_tensor_tensor(out=bo1, in0=wbv_t["h"], scalar=gate128["h"],
                                           in1=bo_t, op0=ALU.mult, op1=ALU.add)
            bo_eff = psmall.tile([128, ND], F32, name="bo_eff", tag="boe", bufs=2)
            nc.vector.scalar_tensor_tensor(out=bo_eff, in0=wbv_t["t"], scalar=gate128["t"],
                                           in1=bo1, op0=ALU.mult, op1=ALU.add)
            bo_eff_box["v"] = bo_eff

        # ---------- projection machinery ----------
        qt = {"h": big.tile([128, HH, SH], BF16, name="qt_h", tag="t_qt2"),
              "t": big.tile([128, HH, SH], BF16, name="qt_t", tag="t_qtp2")}
        kt = {"h": big.tile([128, HH, SH], BF16, name="kt_h", tag="t_ktp"),
              "t": big.tile([128, HH, SH], BF16, name="kt_t", tag="t_ktp2")}
        vv = {"h": big.tile([128, NT, D], F8, name="v_h", tag="t_v"),
              "t": big.tile([128, NT, D], F8, name="v_t", tag="t_v2")}
        KV0 = {"h": 0, "t": SH}

        def kproj(m, ft, couter=False, act_evict=False):
            wk = w_qkv[m][1]
            kp = ps_big2(f"kp_{m}{ft}")
            order = [(p_, c) for c in range(NSQ) for p_ in range(2)] if couter \
                else [(p_, c) for p_ in range(2) for c in range(NSQ)]
            kv0 = KV0[m]
            for p_, c in order:
                nc.tensor.matmul(kp[:, c * 512:(c + 1) * 512],
                                 wk[:, 2 * p_:2 * p_ + 2, ft * 128:(ft + 1) * 128],
                                 xn[:, 2 * p_:2 * p_ + 2, kv0 + c * 512:kv0 + (c + 1) * 512],
                                 start=(p_ == 0), stop=(p_ == 1), perf_mode=DRM)
            if act_evict:
                nc.scalar.activation(kt[m][:, ft, :], kp, ACTF.Identity,
                                     bias=bqk_t[m][:, 4 + ft:5 + ft])
            else:
                nc.vector.tensor_scalar_add(kt[m][:, ft, :], kp, bqk_t[m][:, 4 + ft:5 + ft])

        def qproj(m, ft, couter=False, act_evict=False):
            wq = w_qkv[m][0]
            qp = ps_big2(f"qp_{m}{ft}")
            order = [(p_, c) for c in range(NSQ) for p_ in range(2)] if couter \
                else [(p_, c) for p_ in range(2) for c in range(NSQ)]
            for p_, c in order:
                nc.tensor.matmul(qp[:, c * 512:(c + 1) * 512],
                                 wq[:, 2 * p_:2 * p_ + 2, ft * 128:(ft + 1) * 128],
                                 xn[:, 2 * p_:2 * p_ + 2, c * 512:(c + 1) * 512],
                                 start=(p_ == 0), stop=(p_ == 1), perf_mode=DRM)
            if act_evict:
                nc.scalar.activation(qt[m][:, ft, :], qp, ACTF.Identity,
                                     bias=bqk_t[m][:, ft:ft + 1])
            else:
                nc.vector.tensor_scalar_add(qt[m][:, ft, :], qp, bqk_t[m][:, ft:ft + 1])

        def vproj(m, tt, act_evict=False):
            wv = w_qkv[m][2]
            vp = ps_big2(f"vp_{m}{tt}")
            kv0 = KV0[m]
            for p_ in range(2):
                nc.tensor.matmul(vp[:, 0:512],
                                 xn[:, 2 * p_:2 * p_ + 2, kv0 + tt * 128:kv0 + (tt + 1) * 128],
                                 wv[:, 2 * p_:2 * p_ + 2, :],
                                 start=(p_ == 0), stop=(p_ == 1), perf_mode=DRM)
            if act_evict:
                nc.scalar.activation(vv[m][:, tt, :], vp[:, 0:512], ACTF.Identity)
            else:
                nc.vector.tensor_copy(vv[m][:, tt, :], vp[:, 0:512])

        # module-h projections: K0/Q0 first (head-0 scores are the critical
        # path); V tiles follow and are ready before the first AV matmuls
        kproj("h", 0, couter=True, act_evict=True)
        qproj("h", 0, couter=True, act_evict=True)
        for tt in range(NT // 2):
            vproj("h", tt, act_evict=True)
        # half-1 LN (ssq matmuls fit here; row math + apply-o off-path)
        ssqB1 = ps_big2("lnssq1")
        for pp in range(2):
            for c in range(2):
                nc.tensor.matmul(ssqB1[:, c * 512:(c + 1) * 512], ones128f8_t,
                                 xsq1[pp][:, :, c * 512:(c + 1) * 512],
                                 start=(pp == 0), stop=(pp == 1), perf_mode=DRM)
        rstdB1, meanB1 = row_math(1, sumB1, ssqB1)
        # kv-only half applied on gpsimd (idle engine); needed first by
        # module-t projections two heads into module-h attention
        for d in range(ND):
            t1 = ppt.tile([128, 1024], BF16, name=f"lnt1_{d}", tag="pt")
            nc.gpsimd.tensor_sub(t1, xt_o[:, d, :], meanB1)
            nc.gpsimd.tensor_mul(xn[:, d, SH:S], t1, rstdB1)
        for tt in range(NT // 2, NT):
            vproj("h", tt, act_evict=True)
        # module-t weights stream in while module-h attention runs
        w_qkv["t"] = load_qkv("t", ("t_wa2", "t_wb2", "t_wc2"))

        # ---------- gater sums (emitted after the LN critical path) ----------
        fq_l, fo_l, ad_l = [], [], []
        for d in range(ND):
            junk = ppt.tile([128, 1024], F32, name=f"gjq{d}", tag="pt")
            fq = psmall.tile([128, 1], F32, name=f"fq{d}", tag="gsm")
            nc.scalar.activation(junk, xt_q[:, d, :], ACTF.Copy, accum_out=fq)
            fq_l.append(fq)
        for d in range(ND):
            fo = psmall.tile([128, 1], F32, name=f"fo{d}", tag="gsm")
            nc.vector.reduce_sum(fo, xt_o[:, d, :], axis=mybir.AxisListType.X)
            fo_l.append(fo)
        for d in range(ND):
            ad = psmall.tile([128, 1], F32, name=f"ad{d}", tag="gsm")
            nc.vector.tensor_sub(ad, fq_l[d], fo_l[d])
            ab = psmall.tile([128, 1], F32, name=f"ab{d}", tag="gsm")
            nc.scalar.activation(ab, ad, ACTF.Abs)
            ad_l.append(ab)
        g_in = fq_l + fo_l + ad_l        # raw sums; 1/SH folded into gw1T

        # ---------- attention (software-pipelined inner loop) ----------
        def attn(m, ao, mid=None, post=None):
            for h in range(HH):
                av_ps = ps_av2(f"av_{m}{h}")            # [128, 1024]: sq0 | sq1
                den_ps = ps_av2(f"den_{m}{h}")          # broadcast denominator
                pT = []

                def scores_exp(ttp, h=h, pT=pT):
                    pT2 = ppt.tile([128, 2, 1024], F8, name=f"pT_{m}{h}{ttp}", tag="pt")
                    for i in range(2):
                        tt = 2 * ttp + i
                        sp = ps_big2(f"sp_{m}{h}{tt}")
                        for sq in range(NSQ):
                            nc.tensor.matmul(sp[:, sq * 512:(sq + 1) * 512],
                                             kt[m][:, h, tt * 128:(tt + 1) * 128],
                                             qt[m][:, h, sq * 512:(sq + 1) * 512],
                                             start=True, stop=True)
                        nc.scalar.activation(pT2[:, i, :], sp, ACTF.Exp, scale=INV_SQRT_DH)
                    pT.append(pT2)

                def avden(ttp, h=h, pT=pT):
                    for sq in range(NSQ):
                        nc.tensor.matmul(av_ps[:, sq * 512:(sq + 1) * 512],
                                         vv[m][:, 2 * ttp:2 * ttp + 2, h * 128:(h + 1) * 128],
                                         pT[ttp][:, :, sq * 512:(sq + 1) * 512],
                                         start=(ttp == 0), stop=(ttp == NT // 2 - 1),
                                         perf_mode=DRM)
                    for sq in range(NSQ):
                        nc.tensor.matmul(den_ps[:, sq * 512:(sq + 1) * 512], ones128f8_t,
                                         pT[ttp][:, :, sq * 512:(sq + 1) * 512],
                                         start=(ttp == 0), stop=(ttp == NT // 2 - 1),
                                         perf_mode=DRM)

                scores_exp(0)
                if h + 1 < HH:
                    kproj(m, h + 1)
                    qproj(m, h + 1)
                if h == 0 and mid is not None:
                    mid()
                for ttp in range(1, NT // 2):
                    scores_exp(ttp)
                    avden(ttp - 1)
                avden(NT // 2 - 1)
                rdenB = pbc.tile([128, 1024], F32, name=f"rdenB_{m}{h}", tag="rden", bufs=2)
                nc.vector.reciprocal_approx_fast(rdenB, den_ps)
                # divide straight out of PSUM: ao = (av * gate) * (1/den)
                nc.vector.scalar_tensor_tensor(out=ao[:, h, :], in0=av_ps,
                                               scalar=gate128[m], in1=rdenB,
                                               op0=ALU.mult, op1=ALU.mult)
                if post is not None and h in post:
                    post[h]()

        ao_h = big.tile([128, HH, SH], F8, tag="t_xq")  # reuses xt_q slot

        def post_h2():
            kproj("t", 0)
            qproj("t", 0)

        def post_h3():
            for tt in range(NT):
                vproj("t", tt)

        attn("h", ao_h, mid=gater_tail, post={2: post_h2, 3: post_h3})
        ao_t = big.tile([128, HH, SH], F8, tag="t_ao_t")
        attn("t", ao_t)
        ao = {"h": ao_h, "t": ao_t}
        # prefetch phase 5/6 weights (slots freed by module-h proj / attention)
        w1T_t = big.tile([128, ND, FF], F8, tag="t_xn")
        nc.sync.dma_start(out=w1T_t, in_=re3(w1T_d[:, :]))
        woT_t = {}
        woT_t["h"] = big.tile([128, ND, D], F8, name="wo_h", tag="t_wa")
        nc.sync.dma_start(out=woT_t["h"], in_=re3(woT_d["h"][:, :]))
        woT_t["t"] = big.tile([128, ND, D], F8, name="wo_t", tag="t_wb")
        nc.sync.dma_start(out=woT_t["t"], in_=re3(woT_d["t"][:, :]))

        # ---------- out-proj + residual + FFN, interleaved per s-chunk ----------
        xq2 = big.tile([128, ND, SH], F32, tag="t_qt")   # slot freed by qt_h
        for d in range(ND):
            nc.sync.dma_start(out=xq2[:, d, :],
                              in_=xq32_d[d * 128:(d + 1) * 128, :])
        y32 = big.tile([128, ND, SH], F32, tag="t_y32")
        y_all = big.tile([128, ND, SH], F8, tag="t_xo")
        w2a = big.tile([128, NFF // 2, D], BF16, tag="t_kt")   # ff tiles 0..7
        nc.sync.dma_start(out=w2a, in_=re3(w2T_d[0:FF // 2, :]))
        w2b = big.tile([128, NFF // 2, D], BF16, tag="t_v")    # ff tiles 8..15
        nc.sync.dma_start(out=w2b, in_=re3(w2T_d[FF // 2:FF, :]))

        def w2tile(ff):
            return (w2a, ff) if ff < NFF // 2 else (w2b, ff - NFF // 2)

        for ft in range(ND):
            op = ps_big2(f"op_{ft}")
            for mi, m in enumerate(("h", "t")):
                for pp in range(2):
                    for sq in range(NSQ):
                        nc.tensor.matmul(op[:, sq * 512:(sq + 1) * 512],
                                         woT_t[m][:, 2 * pp:2 * pp + 2, ft * 128:(ft + 1) * 128],
                                         ao[m][:, 2 * pp:2 * pp + 2, sq * 512:(sq + 1) * 512],
                                         start=(mi == 0 and pp == 0), stop=(mi == 1 and pp == 1),
                                         perf_mode=DRM)
            nc.vector.scalar_tensor_tensor(
                out=y32[:, ft, :], in0=op, scalar=bo_eff_box["v"][:, ft:ft + 1],
                in1=xq2[:, ft, :], op0=ALU.add, op1=ALU.add)
            nc.vector.tensor_copy(y_all[:, ft, :], y32[:, ft, :])

        for sq in range(NSQ):
            z_ps = [ps_av2(f"z2_{sq}{i}") for i in range(2)]   # each: ot pair
            for ff in range(NFF):
                hp = ps_big2(f"hp_{sq}{ff}")
                for pp in range(2):
                    nc.tensor.matmul(hp[:, 0:512],
                                     w1T_t[:, 2 * pp:2 * pp + 2, ff * 128:(ff + 1) * 128],
                                     y_all[:, 2 * pp:2 * pp + 2, sq * 512:(sq + 1) * 512],
                                     start=(pp == 0), stop=(pp == 1), perf_mode=DRM)
                hT = ppt.tile([128, 512], BF16, name=f"hT_{sq}{ff}", tag="pth", bufs=4)
                nc.scalar.activation(hT, hp[:, 0:512], ACTF.Gelu, bias=b1_t[:, ff:ff + 1])
                w2t, fl = w2tile(ff)
                for ot in range(ND):
                    nc.tensor.matmul(z_ps[ot // 2][:, (ot % 2) * 512:(ot % 2 + 1) * 512],
                                     w2t[:, fl, ot * 128:(ot + 1) * 128],
                                     hT, start=(ff == 0), stop=(ff == NFF - 1))
            for ot in range(ND):
                z_t = pz.tile([128, 512], BF16, name=f"z_t{sq}{ot}", tag="z")
                nc.vector.scalar_tensor_tensor(
                    out=z_t, in0=z_ps[ot // 2][:, (ot % 2) * 512:(ot % 2 + 1) * 512],
                    scalar=b2_t[:, ot:ot + 1],
                    in1=y32[:, ot, sq * 512:(sq + 1) * 512],
                    op0=ALU.add, op1=ALU.add)
                nc.sync.dma_start(out=zT_d[ot * 128:(ot + 1) * 128, sq * 512:(sq + 1) * 512],
                                  in_=z_t)

        psum.release()
        pz.release()
        psmall.release()
        pbc.release()
        ppt.release()
        big.release()
        const.release()

    lp.__exit__(None, None, None)
    nc.finalize()
    return nc


def _prep_inputs(sequence, g_w1, g_b1, g_w2, g_b2, ln_g, ln_b,
                 homo_in_w, homo_in_b, homo_out_w, homo_out_b,
                 het_in_w, het_in_b, het_out_w, het_out_b,
                 ffn_w1, ffn_b1, ffn_w2, ffn_b2):
    import ml_dtypes
    bf16 = ml_dtypes.bfloat16
    f8 = ml_dtypes.float8_e4m3
    f32 = np.float32
    cc = np.ascontiguousarray

    shared = {}
    shared["gb1"] = cc(np.asarray(g_b1, f32).reshape(HID, 1))
    shared["gw2T"] = cc(np.asarray(g_w2, f32).T)             # [HID, 2]
    shared["gb2n"] = cc(-np.asarray(g_b2, f32).reshape(1, 2))
    ln_g = np.asarray(ln_g, f32)
    ln_b = np.asarray(ln_b, f32)
    for m, in_w, in_b, out_w in (("h", homo_in_w, homo_in_b, homo_out_w),
                                 ("t", het_in_w, het_in_b, het_out_w)):
        in_w = np.asarray(in_w, f32)
        in_b = np.asarray(in_b, f32)
        out_w = np.asarray(out_w, f32)
        wq, wk, wv = in_w[0:D], in_w[D:2 * D], in_w[2 * D:3 * D]
        # fold LN affine into the projections: W' = W*diag(g), b' = b + W@ln_b
        shared[f"wqT_{m}"] = cc((wq * ln_g).T.astype(f8))
        shared[f"wkT_{m}"] = cc((wk * ln_g).T.astype(f8))
        shared[f"wvT_{m}"] = cc((wv * ln_g).T.astype(f8))
        bqk = in_b[0:2 * D].copy()
        bqk[0:D] += wq @ ln_b
        bqk[D:2 * D] += wk @ ln_b
        shared[f"bqk_{m}"] = cc(bqk)
        shared[f"wbv_{m}"] = cc(out_w @ (in_b[2 * D:3 * D] + wv @ ln_b))
        shared[f"woT_{m}"] = cc(out_w.T.astype(f8))
    shared["bo"] = cc(np.asarray(homo_out_b, f32) + np.asarray(het_out_b, f32))
    shared["w1T"] = cc(np.asarray(ffn_w1, f32).T.astype(f8))
    shared["b1"] = cc(np.asarray(ffn_b1, f32))
    shared["w2T"] = cc(np.asarray(ffn_w2, f32).T.astype(bf16))
    shared["b2"] = cc(np.asarray(ffn_b2, f32))
    shared["ones128f8"] = np.ones((128, 2, 128), f8)
    shared["ones128b"] = np.ones((128, 128), bf16)

    # gater MLP consumes raw column sums; fold the 1/SH mean scaling here
    g_w1 = np.asarray(g_w1, f32) / np.float32(SH)
    gw1T = cc(g_w1.T)                                        # [1536, HID]: blocks [f_s|f_b|diff]
    gw1T_swap = cc(np.concatenate([gw1T[D:2 * D], gw1T[0:D], gw1T[2 * D:]], axis=0))

    seq = np.asarray(sequence, f32)
    in_maps = []
    for core in range(NCORE):
        b, p = core // 2, core % 2
        xb = seq[:, b, :]                                    # [S, D]
        xq = xb[p * SH:(p + 1) * SH]
        xo = xb[(1 - p) * SH:(2 - p) * SH]
        xT32 = np.concatenate([xq, xo], axis=0).T            # [D, S]
        m = dict(shared)
        m["xT"] = cc(xT32.astype(bf16))
        m["xq32"] = cc(xT32[:, 0:SH])
        m["gw1T"] = gw1T if p == 0 else gw1T_swap
        in_maps.append(m)
    return in_maps


def kernel(**inputs):
    from concourse.bass_utils import run_bass_kernel_spmd

    if "nc" not in _CACHED:
        _CACHED["nc"] = build_nc()
    nc = _CACHED["nc"]

    in_maps = _prep_inputs(**{k: np.asarray(v) for k, v in inputs.items()})
    core_ids = list(range(NCORE))
    res = run_bass_kernel_spmd(nc, in_maps, core_ids)

    out = np.empty((S, B, D), np.float32)
    for core in range(NCORE):
        b, p = core // 2, core % 2
        out[p * SH:(p + 1) * SH, b, :] = res.results[core]["zT"].astype(np.float32).T
    return out


# revision 18
# speedup vs baseline: 1.2500x; 1.2500x over previous
"""TRN2 Bass kernel for nn_CIE_48052094108098 (sparse_attention).

Model (S=2048, B=4, D=512, H=8 -> HH=4 heads/module, DH=128):
  gates = sigmoid(MLP([mean(x[:1024]), mean(x[1024:]), |diff|]))   (per batch)
  xn = LayerNorm(x)
  homo-MHA: same-half block attention, v gated by gates[:,0]
  het-MHA:  cross-half block attention, v gated by gates[:,1]
  y = x + homo + het;  out = y + FFN(y)   (GELU exact)

Sharding: 8 cores = (batch b) x (query half p). Each core computes the full
output for its (b, half-p) rows; the block masks make each attention dense
over 1024-wide blocks. Zero cross-core communication.

v4: fp8(e4m3) DoubleRow matmuls for QKV projections, AV, softmax
denominators, out-proj and FFN mm1; scores bf16; FFN mm2 bf16 (the hT/w2
fp8 quantization dominated the error budget). Reductions (LN stats,
softmax den) use an all-ones [128,2,128] fp8 stationary so results land in
PSUM broadcast across partitions. LN critical path: half-0 stats first,
half-1 squares on gpsimd, rstd = exp(-0.5*ln(var+eps)) so the scalar
engine stays on one ACT table-set (ln+exp) until GELU. Attention inner
loop is software-pipelined: the next tile-pair's score matmuls are emitted
before the previous pair's AV/den so the PE never waits on the exp.
Module-t projections run inside module-h's attention (PE slack); the
scalar engine never idles between the two modules' exp phases.
"""
import sys

sys.path.insert(0, "/opt/trn_rl_repo")

import numpy as np

S, B, D = 2048, 4, 512
SH = S // 2          # 1024 (half)
HH, DH = 4, 128      # heads per module, head dim
HID = 128            # gater hidden
FF = 4 * D           # 2048
NCORE = 8
ND = D // 128        # 4 feature tiles
NFF = FF // 128      # 16
NT = SH // 128       # 8 t-tiles per kv half
NSQ = SH // 512      # 2 query s-chunks

_CACHED = {}


def build_nc():
    import concourse.mybir as mybir
    import concourse.tile as tile
    from concourse import bacc

    F32 = mybir.dt.float32
    BF16 = mybir.dt.bfloat16
    F8 = mybir.dt.float8e4
    ACTF = mybir.ActivationFunctionType
    ALU = mybir.AluOpType
    DRM = mybir.MatmulPerfMode.DoubleRow

    nc = bacc.Bacc("TRN2", target_bir_lowering=False, debug=False,
                   num_devices=NCORE)

    # ---- DRAM parameters ----
    dp = nc.declare_dram_parameter
    xT_d = dp("xT", [D, S], BF16, isOutput=False)            # cols: [query half | other half]
    ones128b_d = dp("ones128b", [128, 128], BF16, isOutput=False)
    gw1T_d = dp("gw1T", [3 * D, HID], F32, isOutput=False)  # row blocks: f_q, f_o, |diff| (x 1/SH)
    gb1_d = dp("gb1", [HID, 1], F32, isOutput=False)
    gw2T_d = dp("gw2T", [HID, 2], F32, isOutput=False)      # col 0: homo gate, col 1: het
    gb2n_d = dp("gb2n", [1, 2], F32, isOutput=False)        # NEGATED gb2 (for exp-sigmoid)
    wqT_d, wkT_d, wvT_d, bqk_d, wbv_d, woT_d = {}, {}, {}, {}, {}, {}
    for m in ("h", "t"):
        wqT_d[m] = dp(f"wqT_{m}", [D, D], F8, isOutput=False)
        wkT_d[m] = dp(f"wkT_{m}", [D, D], F8, isOutput=False)
        wvT_d[m] = dp(f"wvT_{m}", [D, D], F8, isOutput=False)
        bqk_d[m] = dp(f"bqk_{m}", [2 * D], F32, isOutput=False)   # [bq(512) | bk(512)]
        wbv_d[m] = dp(f"wbv_{m}", [D], F32, isOutput=False)       # out_w @ v_bias
        woT_d[m] = dp(f"woT_{m}", [D, D], F8, isOutput=False)
    bo_d = dp("bo", [D], F32, isOutput=False)                # homo_out_b + het_out_b
    w1T_d = dp("w1T", [D, FF], F8, isOutput=False)
    b1_d = dp("b1", [FF], F32, isOutput=False)
    w2T_d = dp("w2T", [FF, D], BF16, isOutput=False)
    b2_d = dp("b2", [D], F32, isOutput=False)
    ones128f8_d = dp("ones128f8", [128, 2, 128], F8, isOutput=False)
    xq32_d = dp("xq32", [D, SH], F32, isOutput=False)
    zT_d = dp("zT", [D, SH], BF16, isOutput=True)

    INV_SQRT_DH = float(1.0 / np.sqrt(DH))

    def re3(ap):    # [*,N] DRAM -> [128, n, N] partition-tiled view
        return ap.rearrange("(n p) f -> p n f", p=128)

    def rev(ap):    # [N] DRAM vector -> [128, n] column view
        return ap.rearrange("(n p) -> p n", p=128)

    lp = nc.allow_low_precision("fp8/bf16 intermediates: tolerance is 2e-2")
    lp.__enter__()
    with tile.TileContext(nc, pool_alloc_mode="queue") as tc:
        const = tc.alloc_tile_pool(name="const", bufs=1)
        big = tc.alloc_tile_pool(name="big", bufs=1)
        ppt = tc.alloc_tile_pool(name="ppt", bufs=7)
        pbc = tc.alloc_tile_pool(name="pbc", bufs=2)
        psmall = tc.alloc_tile_pool(name="psmall", bufs=16)
        pz = tc.alloc_tile_pool(name="pz", bufs=4)
        psum = tc.alloc_tile_pool(name="psum", bufs=2, space="PSUM")

        def ps_big2(nm):       # [128,1024] two-bank psum (pairs of 512-chunks)
            return psum.tile([128, 1024], F32, name=nm, tag="big2", bufs=2)

        def ps_av2(nm):
            return psum.tile([128, 1024], F32, name=nm, tag="av2", bufs=2)

        # ---------- constants ----------
        # x first: LN is the first consumer of DMA data
        xt_q = big.tile([128, ND, SH], BF16, tag="t_xq")
        xt_o = big.tile([128, ND, SH], BF16, tag="t_xo")
        for d in range(ND):
            nc.sync.dma_start(
                out=xt_q[:, d, :],
                in_=xT_d[d * 128:(d + 1) * 128, 0:SH])
        ones128f8_t = const.tile([128, 2, 128], F8)
        nc.sync.dma_start(out=ones128f8_t, in_=ones128f8_d[:, :, :])
        ones128b_t = const.tile([128, 128], BF16)
        nc.sync.dma_start(out=ones128b_t, in_=ones128b_d[:, :])
        for d in range(ND):
            nc.sync.dma_start(
                out=xt_o[:, d, :],
                in_=xT_d[d * 128:(d + 1) * 128, SH:S])
        gw1T_t = const.tile([128, 12, HID], F32)
        nc.sync.dma_start(out=gw1T_t, in_=gw1T_d[:, :].rearrange("(n p) h -> p n h", p=128))
        gb1_t = const.tile([128, 1], F32)
        nc.sync.dma_start(out=gb1_t, in_=gb1_d[:, :])
        gw2T_t = const.tile([128, 2], F32)
        nc.sync.dma_start(out=gw2T_t, in_=gw2T_d[:, :])
        gb2n_t = const.tile([1, 2], F32)
        nc.sync.dma_start(out=gb2n_t, in_=gb2n_d[:, :])
        bqk_t, wbv_t = {}, {}
        for m in ("h", "t"):
            bqk_t[m] = const.tile([128, 8], F32, name=f"bqk_t_{m}", tag=f"bqk_{m}")
            nc.sync.dma_start(out=bqk_t[m], in_=rev(bqk_d[m][:]))
            wbv_t[m] = const.tile([128, ND], F32, name=f"wbv_t_{m}", tag=f"wbv_{m}")
            nc.sync.dma_start(out=wbv_t[m], in_=rev(wbv_d[m][:]))
        bo_t = const.tile([128, ND], F32)
        nc.sync.dma_start(out=bo_t, in_=rev(bo_d[:]))
        b1_t = const.tile([128, NFF], F32)
        nc.sync.dma_start(out=b1_t, in_=rev(b1_d[:]))
        b2_t = const.tile([128, ND], F32)
        nc.sync.dma_start(out=b2_t, in_=rev(b2_d[:]))
        eps_t = const.tile([128, 1], F32)
        nc.vector.memset(eps_t, 1e-5)
        zero_t = const.tile([128, 1], F32)
        nc.vector.memset(zero_t, 0.0)

        def load_qkv(m, tags):
            wq = big.tile([128, ND, D], F8, name=f"wq_{m}", tag=tags[0])
            nc.sync.dma_start(out=wq, in_=re3(wqT_d[m][:, :]))
            wk = big.tile([128, ND, D], F8, name=f"wk_{m}", tag=tags[1])
            nc.sync.dma_start(out=wk, in_=re3(wkT_d[m][:, :]))
            wv = big.tile([128, ND, D], F8, name=f"wv_{m}", tag=tags[2])
            nc.sync.dma_start(out=wv, in_=re3(wvT_d[m][:, :]))
            return wq, wk, wv

        w_qkv = {"h": load_qkv("h", ("t_wa", "t_wb", "t_wc"))}

        xn = big.tile([128, ND, S], F8, tag="t_xn")

        # ---------- LN half-0 stats (critical path for the projections) ----------
        xsq0 = []
        for pp in range(2):
            sq2 = ppt.tile([128, 2, 1024], F8, name=f"xsq0_{pp}", tag="pt")
            for i in range(2):
                nc.scalar.activation(sq2[:, i, :], xt_q[:, 2 * pp + i, :], ACTF.Square)
            xsq0.append(sq2)
        sumB0 = ps_av2("lnsum0")
        for d in range(ND):
            for c in range(2):
                nc.tensor.matmul(sumB0[:, c * 512:(c + 1) * 512], ones128b_t,
                                 xt_q[:, d, c * 512:(c + 1) * 512],
                                 start=(d == 0), stop=(d == ND - 1))
        ssqB0 = ps_av2("lnssq0")
        for pp in range(2):
            for c in range(2):
                nc.tensor.matmul(ssqB0[:, c * 512:(c + 1) * 512], ones128f8_t,
                                 xsq0[pp][:, :, c * 512:(c + 1) * 512],
                                 start=(pp == 0), stop=(pp == 1), perf_mode=DRM)
        # half-1 squares on gpsimd (off the critical path); sum now, ssq later
        xsq1 = []
        for pp in range(2):
            sq2 = ppt.tile([128, 2, 1024], F8, name=f"xsq1_{pp}", tag="pt")
            for i in range(2):
                nc.gpsimd.tensor_mul(sq2[:, i, :], xt_o[:, 2 * pp + i, :],
                                     xt_o[:, 2 * pp + i, :])
            xsq1.append(sq2)
        sumB1 = ps_big2("lnsum1")
        for d in range(ND):
            for c in range(2):
                nc.tensor.matmul(sumB1[:, c * 512:(c + 1) * 512], ones128b_t,
                                 xt_o[:, d, c * 512:(c + 1) * 512],
                                 start=(d == 0), stop=(d == ND - 1))

        def row_math(hf, sumB, ssqB):
            meanB = pbc.tile([128, 1024], BF16, name=f"meanB{hf}", tag="lnB", bufs=4)
            nc.vector.tensor_scalar_mul(meanB, sumB, 1.0 / D)
            msq = ppt.tile([128, 1024], F32, name=f"msq{hf}", tag="pt")
            nc.vector.tensor_mul(msq, meanB, meanB)
            var = ppt.tile([128, 1024], F32, name=f"var{hf}", tag="pt")
            nc.vector.scalar_tensor_tensor(out=var, in0=ssqB, scalar=1.0 / D, in1=msq,
                                           op0=ALU.mult, op1=ALU.subtract)
            std = ppt.tile([128, 1024], F32, name=f"std{hf}", tag="pt")
            nc.scalar.activation(std, var, ACTF.Sqrt, bias=eps_t)
            rstdB = pbc.tile([128, 1024], F32, name=f"rstdB{hf}", tag="lnB", bufs=4)
            nc.vector.reciprocal_approx_fast(rstdB, std)
            return rstdB, meanB

        rstdB0, meanB0 = row_math(0, sumB0, ssqB0)
        # apply, q half (DVE; chunked so V0-3 can start after chunk 0)
        for c in range(2):
            for d in range(ND):
                t1 = ppt.tile([128, 512], BF16, name=f"lnt0_{d}{c}", tag="pth", bufs=4)
                nc.vector.tensor_sub(t1, xt_q[:, d, c * 512:(c + 1) * 512],
                                     meanB0[:, c * 512:(c + 1) * 512])
                nc.vector.tensor_mul(xn[:, d, c * 512:(c + 1) * 512], t1,
                                     rstdB0[:, c * 512:(c + 1) * 512])

        gates, gate128 = {}, {}
        bo_eff_box = {}

        def gater_tail():
            g1_psum = psum.tile([128, 1], F32, name="g1_psum", tag="big2", bufs=2)
            for i in range(12):
                nc.tensor.matmul(g1_psum, gw1T_t[:, i, :], g_in[i],
                                 start=(i == 0), stop=(i == 11))
            relu_t = psmall.tile([128, 1], F32, tag="gsm")
            nc.vector.scalar_tensor_tensor(out=relu_t, in0=g1_psum, scalar=gb1_t,
                                           in1=zero_t, op0=ALU.add, op1=ALU.max)
            for j, m in enumerate(("h", "t")):
                g2_psum = psum.tile([1, 1], F32, name=f"g2_psum{j}", tag="big2", bufs=2)
                nc.tensor.matmul(g2_psum, gw2T_t[:, j:j + 1], relu_t, start=True, stop=True)
                # sigmoid without an ACT table switch: exp (already loaded) + recip
                e_t = psmall.tile([1, 1], F32, name=f"ge{j}", tag="gsm")
                nc.scalar.activation(e_t, g2_psum, ACTF.Exp, scale=-1.0,
                                     bias=gb2n_t[:, j:j + 1])
                e1_t = psmall.tile([1, 1], F32, name=f"ge1{j}", tag="gsm")
                nc.vector.tensor_scalar_add(e1_t, e_t, 1.0)
                gate = psmall.tile([1, 1], F32, name=f"gate{j}", tag="gsm")
                nc.vector.reciprocal_approx_fast(gate, e1_t)
                gates[m] = gate
                g128 = pbc.tile([128, 1], F32, name=f"g128_{j}", tag="g128", bufs=2)
                nc.gpsimd.partition_broadcast(g128, gate)
                gate128[m] = g128
            bo1 = psmall.tile([128, ND], F32, name="bo1", tag="boe", bufs=2)
            nc.vector.scalar_tensor_tensor(out=bo1, in0=wbv_t["h"], scalar=gate128["h"],
                                           in1=bo_t, op0=ALU.mult, op1=ALU.add)
            bo_eff = psmall.tile([128, ND], F32, name="bo_eff", tag="boe", bufs=2)
            nc.vector.scalar_tensor_tensor(out=bo_eff, in0=wbv_t["t"], scalar=gate128["t"],
                                           in1=bo1, op0=ALU.mult, op1=ALU.add)
            bo_eff_box["v"] = bo_eff

        # ---------- projection machinery ----------
        qt = {"h": big.tile([128, HH, SH], BF16, name="qt_h", tag="t_qt2"),
              "t": big.tile([128, HH, SH], BF16, name="qt_t", tag="t_qtp2")}
        kt = {"h": big.tile([128, HH, SH], BF16, name="kt_h", tag="t_ktp"),
              "t": big.tile([128, HH, SH], BF16, name="kt_t", tag="t_ktp2")}
        vv = {"h": big.tile([128, NT, D], F8, name="v_h", tag="t_v"),
              "t": big.tile([128, NT, D], F8, name="v_t", tag="t_v2")}
        KV0 = {"h": 0, "t": SH}

        def kproj(m, ft, couter=False, act_evict=False):
            wk = w_qkv[m][1]
            kp = ps_big2(f"kp_{m}{ft}")
            order = [(p_, c) for c in range(NSQ) for p_ in range(2)] if couter \
                else [(p_, c) for p_ in range(2) for c in range(NSQ)]
            kv0 = KV0[m]
            for p_, c in order:
                nc.tensor.matmul(kp[:, c * 512:(c + 1) * 512],
                                 wk[:, 2 * p_:2 * p_ + 2, ft * 128:(ft + 1) * 128],
                                 xn[:, 2 * p_:2 * p_ + 2, kv0 + c * 512:kv0 + (c + 1) * 512],
                                 start=(p_ == 0), stop=(p_ == 1), perf_mode=DRM)
            if act_evict:
                nc.scalar.activation(kt[m][:, ft, :], kp, ACTF.Identity,
                                     bias=bqk_t[m][:, 4 + ft:5 + ft])
            else:
                nc.vector.tensor_scalar_add(kt[m][:, ft, :], kp, bqk_t[m][:, 4 + ft:5 + ft])

        def qproj(m, ft, couter=False, act_evict=False):
            wq = w_qkv[m][0]
            qp = ps_big2(f"qp_{m}{ft}")
            order = [(p_, c) for c in range(NSQ) for p_ in range(2)] if couter \
                else [(p_, c) for p_ in range(2) for c in range(NSQ)]
            for p_, c in order:
                nc.tensor.matmul(qp[:, c * 512:(c + 1) * 512],
                                 wq[:, 2 * p_:2 * p_ + 2, ft * 128:(ft + 1) * 128],
                                 xn[:, 2 * p_:2 * p_ + 2, c * 512:(c + 1) * 512],
                                 start=(p_ == 0), stop=(p_ == 1), perf_mode=DRM)
            if act_evict:
                nc.scalar.activation(qt[m][:, ft, :], qp, ACTF.Identity,
                                     bias=bqk_t[m][:, ft:ft + 1])
            else:
                nc.vector.tensor_scalar_add(qt[m][:, ft, :], qp, bqk_t[m][:, ft:ft + 1])

        def vproj(m, tt, act_evict=False):
            wv = w_qkv[m][2]
            vp = ps_big2(f"vp_{m}{tt}")
            kv0 = KV0[m]
            for p_ in range(2):
                nc.tensor.matmul(vp[:, 0:512],
                                 xn[:, 2 * p_:2 * p_ + 2, kv0 + tt * 128:kv0 + (tt + 1) * 128],
                                 wv[:, 2 * p_:2 * p_ + 2, :],
                                 start=(p_ == 0), stop=(p_ == 1), perf_mode=DRM)
            if act_evict:
                nc.scalar.activation(vv[m][:, tt, :], vp[:, 0:512], ACTF.Identity)
            else:
                nc.vector.tensor_copy(vv[m][:, tt, :], vp[:, 0:512])

        # module-h projections: K0/Q0 first (head-0 scores are the critical
        # path); V tiles follow and are ready before the first AV matmuls
        kproj("h", 0, couter=True, act_evict=True)
        qproj("h", 0, couter=True, act_evict=True)
        for tt in range(NT // 2):
            vproj("h", tt, act_evict=True)
        # half-1 LN (ssq matmuls fit here; row math + apply-o off-path)
        ssqB1 = ps_big2("lnssq1")
        for pp in range(2):
            for c in range(2):
                nc.tensor.matmul(ssqB1[:, c * 512:(c + 1) * 512], ones128f8_t,
                                 xsq1[pp][:, :, c * 512:(c + 1) * 512],
                                 start=(pp == 0), stop=(pp == 1), perf_mode=DRM)
        rstdB1, meanB1 = row_math(1, sumB1, ssqB1)
        # kv-only half applied on gpsimd (idle engine); needed first by
        # module-t projections two heads into module-h attention
        for d in range(ND):
            t1 = ppt.tile([128, 1024], BF16, name=f"lnt1_{d}", tag="pt")
            nc.gpsimd.tensor_sub(t1, xt_o[:, d, :], meanB1)
            nc.gpsimd.tensor_mul(xn[:, d, SH:S], t1, rstdB1)
        for tt in range(NT // 2, NT):
            vproj("h", tt, act_evict=True)
        # module-t weights stream in while module-h attention runs
        w_qkv["t"] = load_qkv("t", ("t_wa2", "t_wb2", "t_wc2"))

        # ---------- gater sums (emitted after the LN critical path) ----------
        fq_l, fo_l, ad_l = [], [], []
        for d in range(ND):
            junk = ppt.tile([128, 1024], F32, name=f"gjq{d}", tag="pt")
            fq = psmall.tile([128, 1], F32, name=f"fq{d}", tag="gsm")
            nc.scalar.activation(junk, xt_q[:, d, :], ACTF.Copy, accum_out=fq)
            fq_l.append(fq)
        for d in range(ND):
            fo = psmall.tile([128, 1], F32, name=f"fo{d}", tag="gsm")
            nc.vector.reduce_sum(fo, xt_o[:, d, :], axis=mybir.AxisListType.X)
            fo_l.append(fo)
        for d in range(ND):
            ad = psmall.tile([128, 1], F32, name=f"ad{d}", tag="gsm")
            nc.vector.tensor_sub(ad, fq_l[d], fo_l[d])
            ab = psmall.tile([128, 1], F32, name=f"ab{d}", tag="gsm")
            nc.scalar.activation(ab, ad, ACTF.Abs)
            ad_l.append(ab)
        g_in = fq_l + fo_l + ad_l        # raw sums; 1/SH folded into gw1T

        # ---------- attention (software-pipelined inner loop) ----------
        def attn(m, ao, mid=None, post=None):
            for h in range(HH):
                av_ps = ps_av2(f"av_{m}{h}")            # [128, 1024]: sq0 | sq1
                den_ps = ps_av2(f"den_{m}{h}")          # broadcast denominator
                pT = []

                def scores_exp(ttp, h=h, pT=pT):
                    pT2 = ppt.tile([128, 2, 1024], F8, name=f"pT_{m}{h}{ttp}", tag="pt")
                    for i in range(2):
                        tt = 2 * ttp + i
                        sp = ps_big2(f"sp_{m}{h}{tt}")
                        for sq in range(NSQ):
                            nc.tensor.matmul(sp[:, sq * 512:(sq + 1) * 512],
                                             kt[m][:, h, tt * 128:(tt + 1) * 128],
                                             qt[m][:, h, sq * 512:(sq + 1) * 512],
                                             start=True, stop=True)
                        nc.scalar.activation(pT2[:, i, :], sp, ACTF.Exp, scale=INV_SQRT_DH)
                    pT.append(pT2)

                def avden(ttp, h=h, pT=pT):
                    for sq in range(NSQ):
                        nc.tensor.matmul(av_ps[:, sq * 512:(sq + 1) * 512],
                                         vv[m][:, 2 * ttp:2 * ttp + 2, h * 128:(h + 1) * 128],
                                         pT[ttp][:, :, sq * 512:(sq + 1) * 512],
                                         start=(ttp == 0), stop=(ttp == NT // 2 - 1),
                                         perf_mode=DRM)
                    for sq in range(NSQ):
                        nc.tensor.matmul(den_ps[:, sq * 512:(sq + 1) * 512], ones128f8_t,
                                         pT[ttp][:, :, sq * 512:(sq + 1) * 512],
                                         start=(ttp == 0), stop=(ttp == NT // 2 - 1),
                                         perf_mode=DRM)

                scores_exp(0)
                if h == 0 and mid is not None:
                    mid()
                scores_exp(1)
                avden(0)
                if h + 1 < HH:
                    kproj(m, h + 1)
                scores_exp(2)
                avden(1)
                if h + 1 < HH:
                    qproj(m, h + 1)
                scores_exp(3)
                avden(2)
                avden(3)
                rdenB = pbc.tile([128, 1024], F32, name=f"rdenB_{m}{h}", tag="rden", bufs=2)
                nc.vector.reciprocal_approx_fast(rdenB, den_ps)
                # divide straight out of PSUM: ao = (av * gate) * (1/den)
                nc.vector.scalar_tensor_tensor(out=ao[:, h, :], in0=av_ps,
                                               scalar=gate128[m], in1=rdenB,
                                               op0=ALU.mult, op1=ALU.mult)
                if post is not None and h in post:
                    post[h]()

        ao_h = big.tile([128, HH, SH], F8, tag="t_xq")  # reuses xt_q slot

        def post_h2():
            kproj("t", 0)
            qproj("t", 0)

        def post_h3():
            for tt in range(NT):
                vproj("t", tt)

        attn("h", ao_h, mid=gater_tail, post={2: post_h2, 3: post_h3})
        ao_t = big.tile([128, HH, SH], F8, tag="t_ao_t")
        attn("t", ao_t)
        ao = {"h": ao_h, "t": ao_t}
        # prefetch phase 5/6 weights (slots freed by module-h proj / attention)
        w1T_t = big.tile([128, ND, FF], F8, tag="t_xn")
        nc.sync.dma_start(out=w1T_t, in_=re3(w1T_d[:, :]))
        woT_t = {}
        woT_t["h"] = big.tile([128, ND, D], F8, name="wo_h", tag="t_wa")
        nc.sync.dma_start(out=woT_t["h"], in_=re3(woT_d["h"][:, :]))
        woT_t["t"] = big.tile([128, ND, D], F8, name="wo_t", tag="t_wb")
        nc.sync.dma_start(out=woT_t["t"], in_=re3(woT_d["t"][:, :]))

        # ---------- out-proj + residual + FFN, interleaved per s-chunk ----------
        xq2 = big.tile([128, ND, SH], F32, tag="t_qt")   # slot freed by qt_h
        for d in range(ND):
            nc.sync.dma_start(out=xq2[:, d, :],
                              in_=xq32_d[d * 128:(d + 1) * 128, :])
        y32 = big.tile([128, ND, SH], F32, tag="t_y32")
        y_all = big.tile([128, ND, SH], F8, tag="t_xo")
        w2a = big.tile([128, NFF // 2, D], BF16, tag="t_kt")   # ff tiles 0..7
        nc.sync.dma_start(out=w2a, in_=re3(w2T_d[0:FF // 2, :]))
        w2b = big.tile([128, NFF // 2, D], BF16, tag="t_v")    # ff tiles 8..15
        nc.sync.dma_start(out=w2b, in_=re3(w2T_d[FF // 2:FF, :]))

        def w2tile(ff):
            return (w2a, ff) if ff < NFF // 2 else (w2b, ff - NFF // 2)

        for ft in range(ND):
            op = ps_big2(f"op_{ft}")
            for mi, m in enumerate(("h", "t")):
                for pp in range(2):
                    for sq in range(NSQ):
                        nc.tensor.matmul(op[:, sq * 512:(sq + 1) * 512],
                                         woT_t[m][:, 2 * pp:2 * pp + 2, ft * 128:(ft + 1) * 128],
                                         ao[m][:, 2 * pp:2 * pp + 2, sq * 512:(sq + 1) * 512],
                                         start=(mi == 0 and pp == 0), stop=(mi == 1 and pp == 1),
                                         perf_mode=DRM)
            nc.vector.scalar_tensor_tensor(
                out=y32[:, ft, :], in0=op, scalar=bo_eff_box["v"][:, ft:ft + 1],
                in1=xq2[:, ft, :], op0=ALU.add, op1=ALU.add)
            nc.vector.tensor_copy(y_all[:, ft, :], y32[:, ft, :])

        for sq in range(NSQ):
            z_ps = [ps_av2(f"z2_{sq}{i}") for i in range(2)]   # each: ot pair
            hTs = {}

            def mm1_gelu(ff):
                hp = ps_big2(f"hp_{sq}{ff}")
                for pp in range(2):
                    nc.tensor.matmul(hp[:, 0:512],
                                     w1T_t[:, 2 * pp:2 * pp + 2, ff * 128:(ff + 1) * 128],
                                     y_all[:, 2 * pp:2 * pp + 2, sq * 512:(sq + 1) * 512],
                                     start=(pp == 0), stop=(pp == 1), perf_mode=DRM)
                hT = ppt.tile([128, 512], BF16, name=f"hT_{sq}{ff}", tag="pth", bufs=4)
                nc.scalar.activation(hT, hp[:, 0:512], ACTF.Gelu, bias=b1_t[:, ff:ff + 1])
                hTs[ff] = hT

            def mm2(ff):
                w2t, fl = w2tile(ff)
                hT = hTs.pop(ff)
                for ot in range(ND):
                    nc.tensor.matmul(z_ps[ot // 2][:, (ot % 2) * 512:(ot % 2 + 1) * 512],
                                     w2t[:, fl, ot * 128:(ot + 1) * 128],
                                     hT, start=(ff == 0), stop=(ff == NFF - 1))

            mm1_gelu(0)
            for ff in range(1, NFF):
                mm1_gelu(ff)
                mm2(ff - 1)
            mm2(NFF - 1)
            for ot in range(ND):
                z_t = pz.tile([128, 512], BF16, name=f"z_t{sq}{ot}", tag="z")
                nc.vector.scalar_tensor_tensor(
                    out=z_t, in0=z_ps[ot // 2][:, (ot % 2) * 512:(ot % 2 + 1) * 512],
                    scalar=b2_t[:, ot:ot + 1],
                    in1=y32[:, ot, sq * 512:(sq + 1) * 512],
                    op0=ALU.add, op1=ALU.add)
                nc.sync.dma_start(out=zT_d[ot * 128:(ot + 1) * 128, sq * 512:(sq + 1) * 512],
                                  in_=z_t)

        psum.release()
        pz.release()
        psmall.release()
        pbc.release()
        ppt.release()
        big.release()
        const.release()

    lp.__exit__(None, None, None)
    nc.finalize()
    return nc


def _prep_inputs(sequence, g_w1, g_b1, g_w2, g_b2, ln_g, ln_b,
                 homo_in_w, homo_in_b, homo_out_w, homo_out_b,
                 het_in_w, het_in_b, het_out_w, het_out_b,
                 ffn_w1, ffn_b1, ffn_w2, ffn_b2):
    import ml_dtypes
    bf16 = ml_dtypes.bfloat16
    f8 = ml_dtypes.float8_e4m3
    f32 = np.float32
    cc = np.ascontiguousarray

    shared = {}
    shared["gb1"] = cc(np.asarray(g_b1, f32).reshape(HID, 1))
    shared["gw2T"] = cc(np.asarray(g_w2, f32).T)             # [HID, 2]
    shared["gb2n"] = cc(-np.asarray(g_b2, f32).reshape(1, 2))
    ln_g = np.asarray(ln_g, f32)
    ln_b = np.asarray(ln_b, f32)
    for m, in_w, in_b, out_w in (("h", homo_in_w, homo_in_b, homo_out_w),
                                 ("t", het_in_w, het_in_b, het_out_w)):
        in_w = np.asarray(in_w, f32)
        in_b = np.asarray(in_b, f32)
        out_w = np.asarray(out_w, f32)
        wq, wk, wv = in_w[0:D], in_w[D:2 * D], in_w[2 * D:3 * D]
        # fold LN affine into the projections: W' = W*diag(g), b' = b + W@ln_b
        shared[f"wqT_{m}"] = cc((wq * ln_g).T.astype(f8))
        shared[f"wkT_{m}"] = cc((wk * ln_g).T.astype(f8))
        shared[f"wvT_{m}"] = cc((wv * ln_g).T.astype(f8))
        bqk = in_b[0:2 * D].copy()
        bqk[0:D] += wq @ ln_b
        bqk[D:2 * D] += wk @ ln_b
        shared[f"bqk_{m}"] = cc(bqk)
        shared[f"wbv_{m}"] = cc(out_w @ (in_b[2 * D:3 * D] + wv @ ln_b))
        shared[f"woT_{m}"] = cc(out_w.T.astype(f8))
    shared["bo"] = cc(np.asarray(homo_out_b, f32) + np.asarray(het_out_b, f32))
    shared["w1T"] = cc(np.asarray(ffn_w1, f32).T.astype(f8))
    shared["b1"] = cc(np.asarray(ffn_b1, f32))
    shared["w2T"] = cc(np.asarray(ffn_w2, f32).T.astype(bf16))
    shared["b2"] = cc(np.asarray(ffn_b2, f32))
    shared["ones128f8"] = np.ones((128, 2, 128), f8)
    shared["ones128b"] = np.ones((128, 128), bf16)

    # gater MLP consumes raw column sums; fold the 1/SH mean scaling here
    g_w1 = np.asarray(g_w1, f32) / np.float32(SH)
    gw1T = cc(g_w1.T)                                        # [1536, HID]: blocks [f_s|f_b|diff]
    gw1T_swap = cc(np.concatenate([gw1T[D:2 * D], gw1T[0:D], gw1T[2 * D:]], axis=0))

    seq = np.asarray(sequence, f32)
    in_maps = []
    for core in range(NCORE):
        b, p = core // 2, core % 2
        xb = seq[:, b, :]                                    # [S, D]
        xq = xb[p * SH:(p + 1) * SH]
        xo = xb[(1 - p) * SH:(2 - p) * SH]
        xT32 = np.concatenate([xq, xo], axis=0).T            # [D, S]
        m = dict(shared)
        m["xT"] = cc(xT32.astype(bf16))
        m["xq32"] = cc(xT32[:, 0:SH])
        m["gw1T"] = gw1T if p == 0 else gw1T_swap
        in_maps.append(m)
    return in_maps


def kernel(**inputs):
    from concourse.bass_utils import run_bass_kernel_spmd

    if "nc" not in _CACHED:
        _CACHED["nc"] = build_nc()
    nc = _CACHED["nc"]

    in_maps = _prep_inputs(**{k: np.asarray(v) for k, v in inputs.items()})
    core_ids = list(range(NCORE))
    res = run_bass_kernel_spmd(nc, in_maps, core_ids)

    out = np.empty((S, B, D), np.float32)
    for core in range(NCORE):
        b, p = core // 2, core % 2
        out[p * SH:(p + 1) * SH, b, :] = res.results[core]["zT"].astype(np.float32).T
    return out
